# revision 29
# baseline (speedup 1.0000x reference)
"""Trainium2 Bass kernel for nn_Discriminator (conv-highway discriminator + cosine retrieval).

Model (per reference):
  emb = emb_w[x]                          # [64, 128, 300]
  pred     = branch(emb, conv_w*, hw_w)   # [64, 2] log-softmax
  pred_ref = branch(emb, convr_*, hwr_w)  # only rows 0..15 are used
  values[i] = sum_j cos(pred_ref[j], pred[i]);  out = log(values / values.sum())

Sharding: 80 row-units (64 pred + 16 ref) split 10 per core: core c computes
pred rows 8c..8c+7 and ref rows 2c, 2c+1.  Each core returns RAW logits
[10, 2]; the host does log-softmax + L2-normalize + the tiny cosine-sum and
log normalizer (O(B) work).

On-device pipeline per core (v1, HAM-warm restructure):
  - per-row indirect-DMA gathers ordered ref rows first -> transpose to
    channel-major via REGULAR matmuls against an identity rhs (these count as
    PE activity for the HAM clock gate, unlike transpose-mode)
  - conv-as-matmul (bf16) with the E=300 remainder rows (256:300) shift-packed
    across filter taps into 88/88/44-row K-tiles (88 MMs per position chunk
    instead of 102)
  - phase order follows gather arrival: ref chunk (N=256, needs only gathers
    8,9 + ref weights) right after warm-up, then pred chunk j=0 (gathers 0-3),
    then j=1 (gathers 4-7), so the PE never idles past a HAM window
  - highway (bf16) batch-stationary matmuls interleaved per g in the last
    phase; bias folded as a ones-row matmul
  - epilogue: sigmoid/relu/mix in bf16, transpose back via regular matmuls,
    linear to logits; log-softmax/normalize moved to host
  - single ACT table set (sigmoid_and_others covers sigmoid+relu), preloaded
    by a dummy activation at t=0
fp32 PSUM accumulation throughout; bf16 operands (PE 2.4 GHz HAM clock).
"""

import os
import sys

for _p in ("/opt/trn_rl_repo", "/root/.axon_site/_ro/trn_rl_repo"):
    if os.path.isdir(_p) and _p not in sys.path:
        sys.path.insert(0, _p)

import ml_dtypes
import numpy as np

import concourse.bass as bass
import concourse.mybir as mybir
import concourse.tile as tile
from concourse import bacc
from concourse.bass_utils import run_bass_kernel_spmd

# ---- problem constants (hardcoded per spec) ----
B, REF, L, V, E = 64, 16, 128, 50000, 300
FS = [3, 4, 5]
NF = [300, 300, 400]
F = 1000                      # sum(NF)
NCLS = 2
N_CORES = 8
RPC = 10                      # rows per core: 8 pred + 2 ref
KMAX = 5

F32 = mybir.dt.float32
BF16 = mybir.dt.bfloat16
FP8 = mybir.dt.float8e4
DR = mybir.MatmulPerfMode.DoubleRow
I32 = mybir.dt.int32
AX = mybir.AxisListType
AFT = mybir.ActivationFunctionType
ALU = mybir.AluOpType

# full 128-row contraction chunks; rows 256:300 go through the shift-packed
# remainder tiles RA (shifts 0,1), RB (shifts 2,3), RC (shift 4)
ECH = [(0, 128), (128, 128)]
GCH = [(g * 128, min(128, F - g * 128)) for g in range(8)]
# valid filter-shifts k contributing to feature chunk g (zero-padded weights
# make over-inclusive k harmless)
KSET = [range(3), range(3), range(4), range(4), range(5), range(5), range(5), range(5)]
# per-chunk segments (p0, pm, f): feature sub-ranges belonging to one conv unit
SEG = [
    [(0, 128, 3)], [(0, 128, 3)],
    [(0, 44, 3), (44, 84, 4)],
    [(0, 128, 4)],
    [(0, 88, 4), (88, 40, 5)],
    [(0, 128, 5)], [(0, 128, 5)], [(0, 104, 5)],
]
# remainder tiles used per g: RA+RB always (zero-padded), RC only when f=5
NREM = [2, 2, 2, 2, 3, 3, 3, 3]
# token position chunks: (width, n_rows, out_col0); j=2 is the ref chunk
POS = [(512, 4, 0), (512, 4, 4), (256, 2, 8)]
WJ = [528, 528, 272]          # embT chunk widths (shift overlap, 16B-aligned)
PF = 1024                     # feature dim padded for DoubleRow stride rules

_CACHE = {}


def _build_program():
    nc = bacc.Bacc("TRN2", target_bir_lowering=False, debug=False, num_devices=N_CORES)

    d_idx = nc.dram_tensor("idx", [L, RPC], I32, kind="ExternalInput")
    d_emb = nc.dram_tensor("emb", [V, E], BF16, kind="ExternalInput")
    d_w2 = {br: nc.dram_tensor(f"w2_{br}", [128, KMAX, 2, PF], FP8, kind="ExternalInput")
            for br in "pr"}
    d_rem2 = {br: nc.dram_tensor(f"rem2_{br}", [108, 2, PF], FP8, kind="ExternalInput")
              for br in "pr"}
    d_rc = {br: nc.dram_tensor(f"rc_{br}", [44, PF], FP8, kind="ExternalInput")
            for br in "pr"}
    d_hwt = nc.dram_tensor("hwt", [8, 128, 2 * F], BF16, kind="ExternalInput")
    d_cb = nc.dram_tensor("cb", [128, 8, 2], F32, kind="ExternalInput")
    d_lint = nc.dram_tensor("lint", [128, 8, NCLS], BF16, kind="ExternalInput")
    d_linb = nc.dram_tensor("linb", [1, NCLS], BF16, kind="ExternalInput")
    d_identb = nc.dram_tensor("identb", [128, 128], BF16, kind="ExternalInput")
    d_onesb = nc.dram_tensor("onesb", [1, RPC], BF16, kind="ExternalInput")
    d_hwbb = nc.dram_tensor("hwbb", [1, 2 * F], BF16, kind="ExternalInput")
    d_res = nc.dram_tensor("res", [RPC, NCLS], F32, kind="ExternalOutput")

    with tile.TileContext(nc) as tc:
        _emit(nc, tc, d_idx, d_emb, d_w2, d_rem2, d_rc, d_hwt, d_cb, d_lint,
              d_linb, d_identb, d_onesb, d_hwbb, d_res)
    nc.finalize()
    return nc


def _emit(nc, tc, d_idx, d_emb, d_w2, d_rem2, d_rc, d_hwt, d_cb, d_lint,
          d_linb, d_identb, d_onesb, d_hwbb, d_res):
    STAGE = int(os.environ.get("K_STAGE", "99"))
    from contextlib import ExitStack
    ctx = ExitStack()
    singles = ctx.enter_context(tc.tile_pool(name="singles", bufs=1))
    hwtp = ctx.enter_context(tc.tile_pool(name="hwtp", bufs=1))
    hwx = ctx.enter_context(tc.tile_pool(name="hwx", bufs=8))
    small = ctx.enter_context(tc.tile_pool(name="small", bufs=4))
    ps_tp = ctx.enter_context(tc.tile_pool(name="ps_tp", bufs=1, space="PSUM"))
    ps_cv = ctx.enter_context(tc.tile_pool(name="ps_cv", bufs=3, space="PSUM"))
    ps_hw = ctx.enter_context(tc.tile_pool(name="ps_hw", bufs=4, space="PSUM"))

    # --- ACT table preload: sigmoid_and_others covers sigmoid+relu+copy.
    # Emitting a sigmoid FIRST pins that set so no later table switch occurs.
    scrap = singles.tile([1, 2], F32)
    nc.vector.memset(scrap[:], 0.0)
    scrap2 = singles.tile([1, 2], F32)
    nc.scalar.activation(out=scrap2[:], in_=scrap[:], func=AFT.Sigmoid)

    # --- small constants ---
    idx_sb = singles.tile([L, RPC], I32)
    id_b = singles.tile([128, 128], BF16)
    with tc.high_priority():
        nc.sync.dma_start(out=idx_sb[:], in_=d_idx[:])
        nc.sync.dma_start(out=id_b[:], in_=d_identb[:])

    # --- PE warm-up: dummy matmuls (junk values, scratch psum) to trip the
    # HAM clock gate while the gathers + ref weights stream in.
    junk = singles.tile([128, 512], BF16)
    with tc.high_priority():
        nc.vector.memset(junk[:], 0.0)
        for w_ in range(34):
            warm_ps = ps_cv.tile([128, 512], F32, tag="cv", name=f"warm{w_ % 3}")
            nc.tensor.matmul(out=warm_ps[:], lhsT=id_b[:], rhs=junk[:],
                             start=True, stop=True)

    # --- gathers: one indirect DMA per row-unit, ref rows (8, 9) FIRST so
    # the ref conv phase can start right after warm-up
    G = {}
    for r in [8, 9] + list(range(8)):
        t = singles.tile([L, E], BF16, tag=f"emb_g{r}", name=f"emb_g{r}")
        nc.gpsimd.indirect_dma_start(
            out=t[:], out_offset=None, in_=d_emb[:],
            in_offset=bass.IndirectOffsetOnAxis(ap=idx_sb[:, r:r + 1], axis=0))
        G[r] = t

    # --- conv weights (fp8, DoubleRow pair layout), ref branch first ---
    w2sb = {}    # br -> [128, KMAX*2*PF] (view as [128, k, 2, f])
    rem2sb = {}  # br -> [108, 2*PF]
    rcsb = {}    # br -> [44, PF]
    for br in "rp":
        t = hwtp.tile([128, KMAX * 2 * PF], FP8, tag=f"w2{br}", name=f"w2{br}")
        w2sb[br] = t
        for k in range(KMAX):
            nc.sync.dma_start(out=t[:, k * 2 * PF:(k + 1) * 2 * PF],
                              in_=d_w2[br][:, k, :, :])
        rt = hwtp.tile([108, 2 * PF], FP8, tag=f"rem2{br}", name=f"rem2{br}")
        nc.sync.dma_start(out=rt[:], in_=d_rem2[br][:])
        rem2sb[br] = rt
        rct = hwtp.tile([44, PF], FP8, tag=f"rc{br}", name=f"rc{br}")
        nc.sync.dma_start(out=rct[:], in_=d_rc[br][:])
        rcsb[br] = rct

    # small consts on the scalar queue (parallel to the big sync stream)
    cb_sb = singles.tile([128, 8, 2], F32)
    nc.scalar.dma_start(out=cb_sb[:], in_=d_cb[:])
    lint_sb = singles.tile([128, 8, NCLS], BF16)
    nc.scalar.dma_start(out=lint_sb[:], in_=d_lint[:])
    linb_sb = singles.tile([1, NCLS], BF16)
    nc.scalar.dma_start(out=linb_sb[:], in_=d_linb[:])
    onesb_sb = singles.tile([1, RPC], BF16)
    nc.scalar.dma_start(out=onesb_sb[:], in_=d_onesb[:])
    hwbb_sb = singles.tile([1, 2 * F], BF16)
    nc.scalar.dma_start(out=hwbb_sb[:], in_=d_hwbb[:])

    # highway weights (sync queue, after conv weights; g-major so the per-g
    # highway matmuls unblock progressively; b0/b1 packed in one DMA per g)
    hwt_sb = {}
    for g, (g0, mg) in enumerate(GCH):
        t = hwtp.tile([128, 2 * F], BF16, tag=f"hwt{g}", name=f"hwt{g}")
        hh = 105 if g == 7 else mg
        nc.sync.dma_start(out=t[:hh, :], in_=d_hwt[g, :hh, :])
        hwt_sb[g] = t

    # --- channel-major embeddings (fp8) ---
    # embT01[j]: [128, 2*WJ[j]] -- E-chunk c0 in cols 0:WJ, c1 in WJ:2WJ (the
    # two DoubleRow K-subtiles; subtile stride WJ is 16B-aligned)
    # embRAB[j]: [108, 2*WJ[j]] -- remainder shifts (0,1) / (2,3) packed at
    # partitions 0:44 / 64:108 (rows 44:64 memset: zero weights there, but NaN
    # garbage would poison the accumulation)
    # embRC[j]:  [44, WJ[j]] -- remainder shift 4
    embT01 = [singles.tile([128, 2 * WJ[j]], FP8, tag=f"embT01_{j}", name=f"embT01_{j}")
              for j in range(3)]
    emb2 = [singles.tile([44, WJ[j]], FP8, tag=f"emb2_{j}", name=f"emb2_{j}")
            for j in range(3)]
    embRAB = [singles.tile([108, 2 * WJ[j]], FP8, tag=f"embRAB_{j}", name=f"embRAB_{j}")
              for j in range(3)]
    embRC = [singles.tile([44, WJ[j]], FP8, tag=f"embRC_{j}", name=f"embRC_{j}")
             for j in range(3)]

    def transpose_rows(rows):
        # gather-tile [128 tok, 300] slices -> channel-major via regular
        # matmul; the PSUM->SBUF copy casts f32 -> fp8
        for r in rows:
            src = G[r]
            j, lc = divmod(r * L, 512)
            for c in range(2):
                tp = ps_cv.tile([128, 512], F32, tag="cv", name=f"tpg{r}_{c}")
                nc.tensor.matmul(out=tp[:, :L], lhsT=src[:, 128 * c:128 * (c + 1)],
                                 rhs=id_b[:], start=True, stop=True)
                nc.vector.tensor_copy(out=embT01[j][:, c * WJ[j] + lc:c * WJ[j] + lc + L],
                                      in_=tp[:, :L])
            tp = ps_cv.tile([128, 512], F32, tag="cv", name=f"tpg{r}_2")
            nc.tensor.matmul(out=tp[:44, :L], lhsT=src[:, 256:300],
                             rhs=id_b[:], start=True, stop=True)
            nc.vector.tensor_copy(out=emb2[j][:44, lc:lc + L], in_=tp[:44, :L])

    def fill_and_shift(j):
        # fill the shift-overlap columns with valid (don't-care) data, then
        # build the shift-packed remainder tiles for this position chunk
        w = POS[j][0]
        for c in range(2):
            nc.vector.tensor_copy(out=embT01[j][:, c * WJ[j] + w:c * WJ[j] + w + 8],
                                  in_=embT01[j][:, c * WJ[j]:c * WJ[j] + 8])
        nc.vector.tensor_copy(out=emb2[j][:44, w:w + 8], in_=emb2[j][:44, 0:8])
        nc.vector.memset(embRAB[j][:, :], 0.0)
        for t in range(2):
            for half in range(2):
                sh = 2 * t + half
                nc.vector.tensor_copy(
                    out=embRAB[j][64 * half:64 * half + 44, t * WJ[j]:t * WJ[j] + w + 4 - sh],
                    in_=emb2[j][:44, sh:w + 4])
        nc.vector.tensor_copy(out=embRC[j][:44, 0:w], in_=emb2[j][:44, 4:w + 4])

    pool_g = [singles.tile([128, RPC], F32, tag=f"poolg{g}", name=f"poolg{g}") for g in range(8)]

    def pool_chunk(g, j, psrc):
        g0, mg = GCH[g]
        w, nr, oc = POS[j]
        f_max = max(f for (_, _, f) in SEG[g])
        cnt = L - f_max + 1
        src = psrc[0:mg, :].rearrange("p (r t) -> p r t", r=nr)
        nc.vector.reduce_max(
            out=pool_g[g][0:mg, oc:oc + nr], in_=src[:, :, 0:cnt], axis=AX.X)
        for (p0, pm, f) in SEG[g]:
            if f == f_max:
                continue
            for t in range(cnt, L - f + 1):
                nc.vector.tensor_tensor(
                    out=pool_g[g][0:pm, oc:oc + nr],
                    in0=pool_g[g][0:pm, oc:oc + nr],
                    in1=src[0:pm, :, t], op=ALU.max)

    embT01v = [embT01[j][:].rearrange("p (two w) -> p two w", two=2) for j in range(3)]
    embRABv = [embRAB[j][:].rearrange("p (two w) -> p two w", two=2) for j in range(3)]
    w2v = {br: w2sb[br][:].rearrange("p (k two f) -> p k two f", k=KMAX, two=2)
           for br in "pr"}
    rem2v = {br: rem2sb[br][:].rearrange("p (two f) -> p two f", two=2) for br in "pr"}

    def conv_mms(g, br, outs):
        # outs: list of (psum_ap, j); one DoubleRow MM per K-subtile pair
        g0, mg = GCH[g]
        steps = []
        for k in KSET[g]:
            steps.append((w2v[br][:, k, :, g0:g0 + 128],
                          [embT01v[j][:, :, k:k + POS[j][0]] for j in range(3)], DR))
        steps.append((rem2v[br][:, :, g0:g0 + 128],
                      [embRABv[j][:, :, 0:POS[j][0]] for j in range(3)], DR))
        if NREM[g] == 3:
            steps.append((rcsb[br][:44, g0:g0 + 128],
                          [embRC[j][:44, 0:POS[j][0]] for j in range(3)], None))
        for i, (wap, rhs3, pm) in enumerate(steps):
            st, sp = (i == 0), (i == len(steps) - 1)
            for ps, j in outs:
                nc.tensor.matmul(out=ps, lhsT=wap, rhs=rhs3[j], start=st, stop=sp,
                                 perf_mode=pm)

    def keep_warm(tag_i, n_):
        # gap-filling dummy matmuls: no data deps, so they slot into PE idle
        # windows and keep the HAM clock gate warm across dependency stalls
        for w_ in range(n_):
            kw = ps_tp.tile([128, 128], F32, tag="tp", name=f"kw{tag_i}_{w_}")
            nc.tensor.matmul(out=kw[:, :], lhsT=id_b[:], rhs=junk[:, :128],
                             start=True, stop=True)

    # ================= phase R: ref chunk (j=2) =================
    transpose_rows([8, 9])
    fill_and_shift(2)
    for g, (g0, mg) in enumerate(GCH):
        psu2 = ps_cv.tile([128, 512], F32, tag="cv", name=f"cvR{g}")
        conv_mms(g, "r", [(psu2[:, :256], 2)])
        pool_chunk(g, 2, psu2[:, :256])
        if g == 3:
            transpose_rows(range(4))
            fill_and_shift(0)

    if STAGE <= 1:
        dbg = small.tile([RPC, NCLS], F32, tag="dbg")
        nc.vector.tensor_copy(out=dbg[:], in_=pool_g[0][:RPC, :NCLS])
        nc.sync.dma_start(out=d_res[:], in_=dbg[:])
        ctx.close()
        return

    # ================= phase P0: pred chunk j=0 (gathers 0-3) =================
    for g, (g0, mg) in enumerate(GCH):
        psu0 = ps_cv.tile([128, 512], F32, tag="cv", name=f"cvA{g}")
        conv_mms(g, "p", [(psu0[:, :], 0)])
        pool_chunk(g, 0, psu0)
        if g == 3:
            transpose_rows(range(4, 8))
            fill_and_shift(1)

    # ================= phase P1: pred chunk j=1 (gathers 4-7) =================
    pooledr = [singles.tile([128, RPC], BF16, tag=f"pool{g}", name=f"pool{g}") for g in range(8)]
    # ones row at partition 104 of pooledr[7]: folds the highway bias into the
    # g=7 highway matmul (hwt row 104 of g=7 carries hw_b / hwr_b)
    nc.sync.dma_start(out=pooledr[7][104:105, :], in_=d_onesb[:])
    pT = singles.tile([RPC, F], BF16)
    hps = {}
    NHW = ((0, 512), (512, 488))
    for b in range(2):
        for nh in range(2):
            hps[(b, nh)] = ps_hw.tile([RPC, 512], F32, tag="hp", name=f"hp{b}_{nh}")

    for g, (g0, mg) in enumerate(GCH):
        psu1 = ps_cv.tile([128, 512], F32, tag="cv", name=f"cvB{g}")
        conv_mms(g, "p", [(psu1[:, :], 1)])
        pool_chunk(g, 1, psu1)
        nc.scalar.activation(out=pooledr[g][:mg, 0:8], in_=pool_g[g][:mg, 0:8],
                             func=AFT.Relu, bias=cb_sb[:mg, g, 0:1], scale=1.0)
        nc.scalar.activation(out=pooledr[g][:mg, 8:RPC], in_=pool_g[g][:mg, 8:RPC],
                             func=AFT.Relu, bias=cb_sb[:mg, g, 1:2], scale=1.0)
        # pooled row-major copy for the highway mix epilogue
        tp = ps_tp.tile([128, 128], F32, tag="tp")
        nc.tensor.matmul(out=tp[:RPC, :mg], lhsT=pooledr[g][:mg, :RPC],
                         rhs=id_b[:mg, :mg], start=True, stop=True)
        nc.vector.tensor_copy(out=pT[:, g0:g0 + mg], in_=tp[:RPC, :mg])
        kh = 105 if g == 7 else mg
        for b in range(2):
            for nh, (n0, nw) in enumerate(NHW):
                nc.tensor.matmul(
                    out=hps[(b, nh)][:RPC, :nw], lhsT=pooledr[g][:kh, :RPC],
                    rhs=hwt_sb[g][:kh, b * F + n0:b * F + n0 + nw],
                    start=(g == 0), stop=(g == 7))

    if STAGE <= 2:
        dbg = small.tile([RPC, NCLS], F32, tag="dbg")
        nc.vector.tensor_copy(out=dbg[:], in_=pooledr[0][:RPC, :NCLS])
        nc.sync.dma_start(out=d_res[:], in_=dbg[:])
        ctx.close()
        return

    # ================= epilogue: highway mix + linear =================
    ho_b = [singles.tile([RPC, F], BF16, tag=f"ho{b}", name=f"ho{b}") for b in range(2)]
    hoT = [small.tile([128, RPC], BF16, tag=f"hoT{g % 2}", name=f"hoT{g}") for g in range(8)]
    lps = ps_hw.tile([RPC, 512], F32, tag="hp", name="lps")
    # all four (nh, b) bias+sigmoid+mix chains first (so they run concurrently
    # on ACT/DVE), then the transposes + linear accumulation
    for nh, (n0, nw) in enumerate(NHW):
        for b in range(2):
            hp = hps[(b, nh)]
            s = hwx.tile([RPC, 512], BF16, tag=f"s{nh}{b}", name=f"s{nh}{b}")
            nc.scalar.activation(out=s[:RPC, :nw], in_=hp[:RPC, :nw], func=AFT.Sigmoid)
            rl = hwx.tile([RPC, 512], BF16, tag=f"rl{nh}{b}", name=f"rl{nh}{b}")
            # ho = s*(relu(h) - p) + p ; relu(h)-p fused on DVE, || sigmoid
            nc.vector.scalar_tensor_tensor(out=rl[:RPC, :nw], in0=hp[:RPC, :nw],
                                           scalar=0.0, in1=pT[:RPC, n0:n0 + nw],
                                           op0=ALU.max, op1=ALU.subtract)
            nc.vector.tensor_tensor(out=rl[:RPC, :nw], in0=s[:RPC, :nw],
                                    in1=rl[:RPC, :nw], op=ALU.mult)
            nc.vector.tensor_tensor(out=ho_b[b][:RPC, n0:n0 + nw], in0=rl[:RPC, :nw],
                                    in1=pT[:RPC, n0:n0 + nw], op=ALU.add)
    nc.tensor.matmul(out=lps[:RPC, :NCLS], lhsT=onesb_sb[:1, :RPC],
                     rhs=linb_sb[:1, :], start=True, stop=False)
    keep_warm(0, 14)
    for nh, (n0, nw) in enumerate(NHW):
        for g in range(4 * nh, 4 * nh + 4):
            g0, mg = GCH[g]
            for b, (c0_, c1_) in ((0, (0, 8)), (1, (8, RPC))):
                tp2 = ps_cv.tile([128, 512], F32, tag="cv", name=f"tpho{g}_{b}")
                nc.tensor.matmul(out=tp2[:mg, :RPC], lhsT=ho_b[b][:, g0:g0 + mg],
                                 rhs=id_b[:RPC, :RPC], start=True, stop=True)
                nc.scalar.activation(out=hoT[g][:mg, c0_:c1_], in_=tp2[:mg, c0_:c1_],
                                     func=AFT.Copy)
            nc.tensor.matmul(out=lps[:RPC, :NCLS], lhsT=hoT[g][:mg, :RPC],
                             rhs=lint_sb[:mg, g, :], start=False, stop=(g == 7))
    outn = small.tile([RPC, NCLS], F32, tag="outn")
    nc.vector.tensor_copy(out=outn[:], in_=lps[:RPC, :NCLS])
    nc.sync.dma_start(out=d_res[:], in_=outn[:])
    ctx.close()


def _pack_inputs(inputs):
    """Host-side packing: per-core index slices + shared packed weight arrays."""
    f32 = np.float32
    bf16 = ml_dtypes.bfloat16
    x = np.asarray(inputs["x"]).astype(np.int32)                  # [64, 128]
    wfull = {"p": np.zeros((KMAX, E, F), f32), "r": np.zeros((KMAX, E, F), f32)}
    offs = [0, 300, 600]
    for ui, (f, n) in enumerate(zip(FS, NF)):
        o = offs[ui]
        cw = np.asarray(inputs[f"conv_w{f}"], f32)                # [f, E, n]
        cwr = np.asarray(inputs[f"convr_w{f}"], f32)
        for k in range(f):
            wfull["p"][k, :, o:o + n] = cw[k]
            wfull["r"][k, :, o:o + n] = cwr[k]
    fp8 = ml_dtypes.float8_e4m3
    shared = {}
    for br in "pr":
        w = wfull[br]
        w2 = np.zeros((128, KMAX, 2, PF), f32)
        for k in range(KMAX):
            w2[:, k, 0, :F] = w[k, 0:128, :]
            w2[:, k, 1, :F] = w[k, 128:256, :]
        shared[f"w2_{br}"] = w2.astype(fp8)
        rem2 = np.zeros((108, 2, PF), f32)
        for t_ in range(2):
            for half in range(2):
                sh = 2 * t_ + half
                rem2[64 * half:64 * half + 44, t_, :F] = w[sh, 256:300, :]
        shared[f"rem2_{br}"] = rem2.astype(fp8)
        rc = np.zeros((44, PF), f32)
        rc[:, :F] = w[4, 256:300, :]
        shared[f"rc_{br}"] = rc.astype(fp8)
    cbf = np.stack([
        np.concatenate([np.asarray(inputs[f"conv_b{f}"], f32) for f in FS]),
        np.concatenate([np.asarray(inputs[f"convr_b{f}"], f32) for f in FS]),
    ], axis=1)                                                    # [1000, 2]
    cb = np.zeros((128, 8, 2), f32)
    lintf = np.asarray(inputs["lin_w"], f32).T                    # [1000, 2]
    lint = np.zeros((128, 8, NCLS), f32)
    for g in range(8):
        mg = min(128, F - g * 128)
        cb[:mg, g, :] = cbf[g * 128:g * 128 + mg, :]
        lint[:mg, g, :] = lintf[g * 128:g * 128 + mg, :]
    hwtT = np.stack([np.asarray(inputs["hw_w"], f32).T,
                     np.asarray(inputs["hwr_w"], f32).T])          # [2, F, F]
    hwt = np.zeros((8, 128, 2 * F), f32)
    for g in range(8):
        mg = min(128, F - g * 128)
        hwt[g, :mg, :F] = hwtT[0, g * 128:g * 128 + mg, :]
        hwt[g, :mg, F:] = hwtT[1, g * 128:g * 128 + mg, :]
    hwt[7, 104, :F] = np.asarray(inputs["hw_b"], f32)
    hwt[7, 104, F:] = np.asarray(inputs["hwr_b"], f32)
    hwt = hwt.astype(bf16)
    hwb = np.stack([np.asarray(inputs["hw_b"], f32),
                    np.asarray(inputs["hwr_b"], f32)])            # [2, 1000]
    shared.update(
        emb=np.ascontiguousarray(np.asarray(inputs["emb_w"], f32)).astype(bf16),
        hwt=hwt, cb=cb,
        lint=lint.astype(bf16),
        linb=np.asarray(inputs["lin_b"], f32).reshape(1, NCLS).astype(bf16),
        identb=np.eye(128, dtype=f32).astype(bf16),
        onesb=np.ones((1, RPC), bf16),
        hwbb=hwb.reshape(1, 2 * F).astype(bf16))
    in_maps = []
    for c in range(N_CORES):
        rows = list(range(8 * c, 8 * c + 8)) + [2 * c, 2 * c + 1]
        idx = np.ascontiguousarray(x[rows].T)                     # [128, 10]
        in_maps.append(dict(idx=idx, **shared))
    return in_maps


def run_cores(inputs, trace=False, **kw):
    """Compile (cached) and run on 8 cores; returns (per-core results, BassKernelResults)."""
    if "nc" not in _CACHE:
        _CACHE["nc"] = _build_program()
    nc = _CACHE["nc"]
    in_maps = _pack_inputs(inputs)
    res = run_bass_kernel_spmd(nc, in_maps, list(range(N_CORES)), trace=trace, **kw)
    return res.results, res


def combine(results) -> np.ndarray:
    """Host epilogue: per-row log-softmax + L2-normalize, then the cosine sum."""
    logits = np.concatenate([results[c]["res"] for c in range(N_CORES)]).astype(np.float64)
    m = logits.max(axis=1, keepdims=True)
    ls = m + np.log(np.exp(logits - m).sum(axis=1, keepdims=True))
    pred = logits - ls                                            # [80, 2]
    n = np.maximum(np.linalg.norm(pred, axis=1, keepdims=True), 1e-8)
    pn = pred / n
    is_pred = np.tile([True] * 8 + [False] * 2, N_CORES)
    p, r = pn[is_pred], pn[~is_pred]
    values = p @ r.sum(axis=0)
    return np.log(values / values.sum()).astype(np.float32)


def kernel(**inputs) -> np.ndarray:
    results, _ = run_cores(inputs)
    return combine(results)


# revision 30
# speedup vs baseline: 1.0150x; 1.0150x over previous
"""Trainium2 Bass kernel for nn_Discriminator (conv-highway discriminator + cosine retrieval).

Model (per reference):
  emb = emb_w[x]                          # [64, 128, 300]
  pred     = branch(emb, conv_w*, hw_w)   # [64, 2] log-softmax
  pred_ref = branch(emb, convr_*, hwr_w)  # only rows 0..15 are used
  values[i] = sum_j cos(pred_ref[j], pred[i]);  out = log(values / values.sum())

Sharding: 80 row-units (64 pred + 16 ref) split 10 per core: core c computes
pred rows 8c..8c+7 and ref rows 2c, 2c+1.  Each core returns RAW logits
[10, 2]; the host does log-softmax + L2-normalize + the tiny cosine-sum and
log normalizer (O(B) work).

On-device pipeline per core (v1, HAM-warm restructure):
  - per-row indirect-DMA gathers ordered ref rows first -> transpose to
    channel-major via REGULAR matmuls against an identity rhs (these count as
    PE activity for the HAM clock gate, unlike transpose-mode)
  - conv-as-matmul (bf16) with the E=300 remainder rows (256:300) shift-packed
    across filter taps into 88/88/44-row K-tiles (88 MMs per position chunk
    instead of 102)
  - phase order follows gather arrival: ref chunk (N=256, needs only gathers
    8,9 + ref weights) right after warm-up, then pred chunk j=0 (gathers 0-3),
    then j=1 (gathers 4-7), so the PE never idles past a HAM window
  - highway (bf16) batch-stationary matmuls interleaved per g in the last
    phase; bias folded as a ones-row matmul
  - epilogue: sigmoid/relu/mix in bf16, transpose back via regular matmuls,
    linear to logits; log-softmax/normalize moved to host
  - single ACT table set (sigmoid_and_others covers sigmoid+relu), preloaded
    by a dummy activation at t=0
fp32 PSUM accumulation throughout; bf16 operands (PE 2.4 GHz HAM clock).
"""

import os
import sys

for _p in ("/opt/trn_rl_repo", "/root/.axon_site/_ro/trn_rl_repo"):
    if os.path.isdir(_p) and _p not in sys.path:
        sys.path.insert(0, _p)

import ml_dtypes
import numpy as np

import concourse.bass as bass
import concourse.mybir as mybir
import concourse.tile as tile
from concourse import bacc
from concourse.bass_utils import run_bass_kernel_spmd

# ---- problem constants (hardcoded per spec) ----
B, REF, L, V, E = 64, 16, 128, 50000, 300
FS = [3, 4, 5]
NF = [300, 300, 400]
F = 1000                      # sum(NF)
NCLS = 2
N_CORES = 8
RPC = 10                      # rows per core: 8 pred + 2 ref
KMAX = 5

F32 = mybir.dt.float32
BF16 = mybir.dt.bfloat16
FP8 = mybir.dt.float8e4
DR = mybir.MatmulPerfMode.DoubleRow
I32 = mybir.dt.int32
AX = mybir.AxisListType
AFT = mybir.ActivationFunctionType
ALU = mybir.AluOpType

# full 128-row contraction chunks; rows 256:300 go through the shift-packed
# remainder tiles RA (shifts 0,1), RB (shifts 2,3), RC (shift 4)
ECH = [(0, 128), (128, 128)]
GCH = [(g * 128, min(128, F - g * 128)) for g in range(8)]
# valid filter-shifts k contributing to feature chunk g (zero-padded weights
# make over-inclusive k harmless)
KSET = [range(3), range(3), range(4), range(4), range(5), range(5), range(5), range(5)]
# per-chunk segments (p0, pm, f): feature sub-ranges belonging to one conv unit
SEG = [
    [(0, 128, 3)], [(0, 128, 3)],
    [(0, 44, 3), (44, 84, 4)],
    [(0, 128, 4)],
    [(0, 88, 4), (88, 40, 5)],
    [(0, 128, 5)], [(0, 128, 5)], [(0, 104, 5)],
]
# remainder tiles used per g: RA+RB always (zero-padded), RC only when f=5
NREM = [2, 2, 2, 2, 3, 3, 3, 3]
# token position chunks: (width, n_rows, out_col0); j=2 is the ref chunk
POS = [(512, 4, 0), (512, 4, 4), (256, 2, 8)]
WJ = [528, 528, 272]          # embT chunk widths (shift overlap, 16B-aligned)
PF = 1024                     # feature dim padded for DoubleRow stride rules

_CACHE = {}


def _build_program():
    nc = bacc.Bacc("TRN2", target_bir_lowering=False, debug=False, num_devices=N_CORES)

    d_idx = nc.dram_tensor("idx", [L, RPC], I32, kind="ExternalInput")
    d_emb = nc.dram_tensor("emb", [V, E], BF16, kind="ExternalInput")
    d_w2 = {br: nc.dram_tensor(f"w2_{br}", [128, KMAX, 2, PF], FP8, kind="ExternalInput")
            for br in "pr"}
    d_rem2 = {br: nc.dram_tensor(f"rem2_{br}", [108, 2, PF], FP8, kind="ExternalInput")
              for br in "pr"}
    d_rc = {br: nc.dram_tensor(f"rc_{br}", [44, PF], FP8, kind="ExternalInput")
            for br in "pr"}
    d_hwt = nc.dram_tensor("hwt", [8, 128, 2 * F], BF16, kind="ExternalInput")
    d_cb = nc.dram_tensor("cb", [128, 8, 2], F32, kind="ExternalInput")
    d_lint = nc.dram_tensor("lint", [128, 8, NCLS], BF16, kind="ExternalInput")
    d_linb = nc.dram_tensor("linb", [1, NCLS], BF16, kind="ExternalInput")
    d_identb = nc.dram_tensor("identb", [128, 128], BF16, kind="ExternalInput")
    d_onesb = nc.dram_tensor("onesb", [1, RPC], BF16, kind="ExternalInput")
    d_hwbb = nc.dram_tensor("hwbb", [1, 2 * F], BF16, kind="ExternalInput")
    d_res = nc.dram_tensor("res", [RPC, NCLS], F32, kind="ExternalOutput")

    with tile.TileContext(nc) as tc:
        _emit(nc, tc, d_idx, d_emb, d_w2, d_rem2, d_rc, d_hwt, d_cb, d_lint,
              d_linb, d_identb, d_onesb, d_hwbb, d_res)
    nc.finalize()
    return nc


def _emit(nc, tc, d_idx, d_emb, d_w2, d_rem2, d_rc, d_hwt, d_cb, d_lint,
          d_linb, d_identb, d_onesb, d_hwbb, d_res):
    STAGE = int(os.environ.get("K_STAGE", "99"))
    from contextlib import ExitStack
    ctx = ExitStack()
    singles = ctx.enter_context(tc.tile_pool(name="singles", bufs=1))
    hwtp = ctx.enter_context(tc.tile_pool(name="hwtp", bufs=1))
    hwx = ctx.enter_context(tc.tile_pool(name="hwx", bufs=8))
    small = ctx.enter_context(tc.tile_pool(name="small", bufs=4))
    ps_tp = ctx.enter_context(tc.tile_pool(name="ps_tp", bufs=1, space="PSUM"))
    ps_cv = ctx.enter_context(tc.tile_pool(name="ps_cv", bufs=3, space="PSUM"))
    ps_hw = ctx.enter_context(tc.tile_pool(name="ps_hw", bufs=4, space="PSUM"))

    # --- ACT table preload: sigmoid_and_others covers sigmoid+relu+copy.
    # Emitting a sigmoid FIRST pins that set so no later table switch occurs.
    scrap = singles.tile([1, 2], F32)
    nc.vector.memset(scrap[:], 0.0)
    scrap2 = singles.tile([1, 2], F32)
    nc.scalar.activation(out=scrap2[:], in_=scrap[:], func=AFT.Sigmoid)

    # --- small constants ---
    idx_sb = singles.tile([L, RPC], I32)
    id_b = singles.tile([128, 128], BF16)
    with tc.high_priority():
        nc.sync.dma_start(out=idx_sb[:], in_=d_idx[:])
        nc.sync.dma_start(out=id_b[:], in_=d_identb[:])

    # --- PE warm-up: dummy matmuls (junk values, scratch psum) to trip the
    # HAM clock gate while the gathers + ref weights stream in.
    junk = singles.tile([128, 512], BF16)
    with tc.high_priority():
        nc.vector.memset(junk[:], 0.0)
        for w_ in range(34):
            warm_ps = ps_cv.tile([128, 512], F32, tag="cv", name=f"warm{w_ % 3}")
            nc.tensor.matmul(out=warm_ps[:], lhsT=id_b[:], rhs=junk[:],
                             start=True, stop=True)

    # --- gathers: one indirect DMA per row-unit, ref rows (8, 9) FIRST so
    # the ref conv phase can start right after warm-up
    G = {}
    for r in [8, 9] + list(range(8)):
        t = singles.tile([L, E], BF16, tag=f"emb_g{r}", name=f"emb_g{r}")
        nc.gpsimd.indirect_dma_start(
            out=t[:], out_offset=None, in_=d_emb[:],
            in_offset=bass.IndirectOffsetOnAxis(ap=idx_sb[:, r:r + 1], axis=0))
        G[r] = t

    # --- conv weights (fp8, DoubleRow pair layout), ref branch first ---
    w2sb = {}    # br -> [128, KMAX*2*PF] (view as [128, k, 2, f])
    rem2sb = {}  # br -> [108, 2*PF]
    rcsb = {}    # br -> [44, PF]
    for br in "rp":
        t = hwtp.tile([128, KMAX * 2 * PF], FP8, tag=f"w2{br}", name=f"w2{br}")
        w2sb[br] = t
        for k in range(KMAX):
            nc.sync.dma_start(out=t[:, k * 2 * PF:(k + 1) * 2 * PF],
                              in_=d_w2[br][:, k, :, :])
        rt = hwtp.tile([108, 2 * PF], FP8, tag=f"rem2{br}", name=f"rem2{br}")
        nc.sync.dma_start(out=rt[:], in_=d_rem2[br][:])
        rem2sb[br] = rt
        rct = hwtp.tile([44, PF], FP8, tag=f"rc{br}", name=f"rc{br}")
        nc.sync.dma_start(out=rct[:], in_=d_rc[br][:])
        rcsb[br] = rct

    # small consts on the scalar queue (parallel to the big sync stream)
    cb_sb = singles.tile([128, 8, 2], F32)
    nc.scalar.dma_start(out=cb_sb[:], in_=d_cb[:])
    lint_sb = singles.tile([128, 8, NCLS], BF16)
    nc.scalar.dma_start(out=lint_sb[:], in_=d_lint[:])
    linb_sb = singles.tile([1, NCLS], BF16)
    nc.scalar.dma_start(out=linb_sb[:], in_=d_linb[:])
    onesb_sb = singles.tile([1, RPC], BF16)
    nc.scalar.dma_start(out=onesb_sb[:], in_=d_onesb[:])
    hwbb_sb = singles.tile([1, 2 * F], BF16)
    nc.scalar.dma_start(out=hwbb_sb[:], in_=d_hwbb[:])

    # highway weights (sync queue, after conv weights; g-major so the per-g
    # highway matmuls unblock progressively; b0/b1 packed in one DMA per g)
    hwt_sb = {}
    for g, (g0, mg) in enumerate(GCH):
        t = hwtp.tile([128, 2 * F], BF16, tag=f"hwt{g}", name=f"hwt{g}")
        hh = 105 if g == 7 else mg
        nc.sync.dma_start(out=t[:hh, :], in_=d_hwt[g, :hh, :])
        hwt_sb[g] = t

    # --- channel-major embeddings (fp8) ---
    # embT01[j]: [128, 2*WJ[j]] -- E-chunk c0 in cols 0:WJ, c1 in WJ:2WJ (the
    # two DoubleRow K-subtiles; subtile stride WJ is 16B-aligned)
    # embRAB[j]: [108, 2*WJ[j]] -- remainder shifts (0,1) / (2,3) packed at
    # partitions 0:44 / 64:108 (rows 44:64 memset: zero weights there, but NaN
    # garbage would poison the accumulation)
    # embRC[j]:  [44, WJ[j]] -- remainder shift 4
    embT01 = [singles.tile([128, 2 * WJ[j]], FP8, tag=f"embT01_{j}", name=f"embT01_{j}")
              for j in range(3)]
    emb2 = [singles.tile([44, WJ[j]], FP8, tag=f"emb2_{j}", name=f"emb2_{j}")
            for j in range(3)]
    embRAB = [singles.tile([108, 2 * WJ[j]], FP8, tag=f"embRAB_{j}", name=f"embRAB_{j}")
              for j in range(3)]
    embRC = [singles.tile([44, WJ[j]], FP8, tag=f"embRC_{j}", name=f"embRC_{j}")
             for j in range(3)]

    for j_ in range(3):
        nc.vector.memset(embRAB[j_][:, :], 0.0)

    def transpose_rows(rows):
        # gather-tile [128 tok, 300] slices -> channel-major via regular
        # matmul; the PSUM->SBUF copy casts f32 -> fp8
        for r in rows:
            src = G[r]
            j, lc = divmod(r * L, 512)
            for c in range(2):
                tp = ps_cv.tile([128, 512], F32, tag="cv", name=f"tpg{r}_{c}")
                nc.tensor.matmul(out=tp[:, :L], lhsT=src[:, 128 * c:128 * (c + 1)],
                                 rhs=id_b[:], start=True, stop=True)
                nc.vector.tensor_copy(out=embT01[j][:, c * WJ[j] + lc:c * WJ[j] + lc + L],
                                      in_=tp[:, :L])
            tp = ps_cv.tile([128, 512], F32, tag="cv", name=f"tpg{r}_2")
            nc.tensor.matmul(out=tp[:44, :L], lhsT=src[:, 256:300],
                             rhs=id_b[:], start=True, stop=True)
            nc.vector.tensor_copy(out=emb2[j][:44, lc:lc + L], in_=tp[:44, :L])

    def fill_and_shift(j):
        # fill the shift-overlap columns with valid (don't-care) data, then
        # build the shift-packed remainder tiles for this position chunk
        w = POS[j][0]
        for c in range(2):
            nc.vector.tensor_copy(out=embT01[j][:, c * WJ[j] + w:c * WJ[j] + w + 8],
                                  in_=embT01[j][:, c * WJ[j]:c * WJ[j] + 8])
        nc.vector.tensor_copy(out=emb2[j][:44, w:w + 8], in_=emb2[j][:44, 0:8])
        for t in range(2):
            for half in range(2):
                sh = 2 * t + half
                nc.vector.tensor_copy(
                    out=embRAB[j][64 * half:64 * half + 44, t * WJ[j]:t * WJ[j] + w + 4 - sh],
                    in_=emb2[j][:44, sh:w + 4])
        nc.vector.tensor_copy(out=embRC[j][:44, 0:w], in_=emb2[j][:44, 4:w + 4])

    pool_g = [singles.tile([128, RPC], F32, tag=f"poolg{g}", name=f"poolg{g}") for g in range(8)]

    def pool_chunk(g, j, psrc):
        g0, mg = GCH[g]
        w, nr, oc = POS[j]
        f_max = max(f for (_, _, f) in SEG[g])
        cnt = L - f_max + 1
        src = psrc[0:mg, :].rearrange("p (r t) -> p r t", r=nr)
        nc.vector.reduce_max(
            out=pool_g[g][0:mg, oc:oc + nr], in_=src[:, :, 0:cnt], axis=AX.X)
        for (p0, pm, f) in SEG[g]:
            if f == f_max:
                continue
            for t in range(cnt, L - f + 1):
                nc.vector.tensor_tensor(
                    out=pool_g[g][0:pm, oc:oc + nr],
                    in0=pool_g[g][0:pm, oc:oc + nr],
                    in1=src[0:pm, :, t], op=ALU.max)

    embT01v = [embT01[j][:].rearrange("p (two w) -> p two w", two=2) for j in range(3)]
    embRABv = [embRAB[j][:].rearrange("p (two w) -> p two w", two=2) for j in range(3)]
    w2v = {br: w2sb[br][:].rearrange("p (k two f) -> p k two f", k=KMAX, two=2)
           for br in "pr"}
    rem2v = {br: rem2sb[br][:].rearrange("p (two f) -> p two f", two=2) for br in "pr"}

    def conv_mms(g, br, outs):
        # outs: list of (psum_ap, j); one DoubleRow MM per K-subtile pair
        g0, mg = GCH[g]
        steps = []
        for k in KSET[g]:
            steps.append((w2v[br][:, k, :, g0:g0 + 128],
                          [embT01v[j][:, :, k:k + POS[j][0]] for j in range(3)], DR))
        steps.append((rem2v[br][:, :, g0:g0 + 128],
                      [embRABv[j][:, :, 0:POS[j][0]] for j in range(3)], DR))
        if NREM[g] == 3:
            steps.append((rcsb[br][:44, g0:g0 + 128],
                          [embRC[j][:44, 0:POS[j][0]] for j in range(3)], None))
        for i, (wap, rhs3, pm) in enumerate(steps):
            st, sp = (i == 0), (i == len(steps) - 1)
            for ps, j in outs:
                nc.tensor.matmul(out=ps, lhsT=wap, rhs=rhs3[j], start=st, stop=sp,
                                 perf_mode=pm)

    def keep_warm(tag_i, n_):
        # gap-filling dummy matmuls: no data deps, so they slot into PE idle
        # windows and keep the HAM clock gate warm across dependency stalls
        for w_ in range(n_):
            kw = ps_tp.tile([128, 128], F32, tag="tp", name=f"kw{tag_i}_{w_}")
            nc.tensor.matmul(out=kw[:, :], lhsT=id_b[:], rhs=junk[:, :128],
                             start=True, stop=True)

    # ================= phase R: ref chunk (j=2) =================
    transpose_rows([8, 9])
    fill_and_shift(2)
    for g, (g0, mg) in enumerate(GCH):
        psu2 = ps_cv.tile([128, 512], F32, tag="cv", name=f"cvR{g}")
        conv_mms(g, "r", [(psu2[:, :256], 2)])
        pool_chunk(g, 2, psu2[:, :256])
        if g == 3:
            transpose_rows(range(4))
            fill_and_shift(0)

    if STAGE <= 1:
        dbg = small.tile([RPC, NCLS], F32, tag="dbg")
        nc.vector.tensor_copy(out=dbg[:], in_=pool_g[0][:RPC, :NCLS])
        nc.sync.dma_start(out=d_res[:], in_=dbg[:])
        ctx.close()
        return

    # ================= phase P0: pred chunk j=0 (gathers 0-3) =================
    for g, (g0, mg) in enumerate(GCH):
        psu0 = ps_cv.tile([128, 512], F32, tag="cv", name=f"cvA{g}")
        conv_mms(g, "p", [(psu0[:, :], 0)])
        pool_chunk(g, 0, psu0)
        if g == 3:
            transpose_rows(range(4, 8))
            fill_and_shift(1)

    # ================= phase P1: pred chunk j=1 (gathers 4-7) =================
    pooledr = [singles.tile([128, RPC], BF16, tag=f"pool{g}", name=f"pool{g}") for g in range(8)]
    # ones row at partition 104 of pooledr[7]: folds the highway bias into the
    # g=7 highway matmul (hwt row 104 of g=7 carries hw_b / hwr_b)
    nc.sync.dma_start(out=pooledr[7][104:105, :], in_=d_onesb[:])
    pT = singles.tile([RPC, F], BF16)
    hps = {}
    NHW = ((0, 512), (512, 488))
    for b in range(2):
        for nh in range(2):
            hps[(b, nh)] = ps_hw.tile([RPC, 512], F32, tag="hp", name=f"hp{b}_{nh}")

    for g, (g0, mg) in enumerate(GCH):
        psu1 = ps_cv.tile([128, 512], F32, tag="cv", name=f"cvB{g}")
        conv_mms(g, "p", [(psu1[:, :], 1)])
        pool_chunk(g, 1, psu1)
        nc.scalar.activation(out=pooledr[g][:mg, 0:8], in_=pool_g[g][:mg, 0:8],
                             func=AFT.Relu, bias=cb_sb[:mg, g, 0:1], scale=1.0)
        nc.scalar.activation(out=pooledr[g][:mg, 8:RPC], in_=pool_g[g][:mg, 8:RPC],
                             func=AFT.Relu, bias=cb_sb[:mg, g, 1:2], scale=1.0)
        # pooled row-major copy for the highway mix epilogue
        tp = ps_tp.tile([128, 128], F32, tag="tp")
        nc.tensor.matmul(out=tp[:RPC, :mg], lhsT=pooledr[g][:mg, :RPC],
                         rhs=id_b[:mg, :mg], start=True, stop=True)
        nc.vector.tensor_copy(out=pT[:, g0:g0 + mg], in_=tp[:RPC, :mg])
        kh = 105 if g == 7 else mg
        for b in range(2):
            for nh, (n0, nw) in enumerate(NHW):
                nc.tensor.matmul(
                    out=hps[(b, nh)][:RPC, :nw], lhsT=pooledr[g][:kh, :RPC],
                    rhs=hwt_sb[g][:kh, b * F + n0:b * F + n0 + nw],
                    start=(g == 0), stop=(g == 7))

    if STAGE <= 2:
        dbg = small.tile([RPC, NCLS], F32, tag="dbg")
        nc.vector.tensor_copy(out=dbg[:], in_=pooledr[0][:RPC, :NCLS])
        nc.sync.dma_start(out=d_res[:], in_=dbg[:])
        ctx.close()
        return

    # ================= epilogue: highway mix + linear =================
    ho_b = [singles.tile([RPC, F], BF16, tag=f"ho{b}", name=f"ho{b}") for b in range(2)]
    hoT = [small.tile([128, RPC], BF16, tag=f"hoT{g % 2}", name=f"hoT{g}") for g in range(8)]
    lps = ps_hw.tile([RPC, 512], F32, tag="hp", name="lps")
    # all four (nh, b) bias+sigmoid+mix chains first (so they run concurrently
    # on ACT/DVE), then the transposes + linear accumulation
    for nh, (n0, nw) in enumerate(NHW):
        for b in range(2):
            hp = hps[(b, nh)]
            s = hwx.tile([RPC, 512], BF16, tag=f"s{nh}{b}", name=f"s{nh}{b}")
            nc.scalar.activation(out=s[:RPC, :nw], in_=hp[:RPC, :nw], func=AFT.Sigmoid)
            rl = hwx.tile([RPC, 512], BF16, tag=f"rl{nh}{b}", name=f"rl{nh}{b}")
            # ho = s*(relu(h) - p) + p ; relu(h)-p fused on DVE, || sigmoid
            nc.vector.scalar_tensor_tensor(out=rl[:RPC, :nw], in0=hp[:RPC, :nw],
                                           scalar=0.0, in1=pT[:RPC, n0:n0 + nw],
                                           op0=ALU.max, op1=ALU.subtract)
            nc.vector.tensor_tensor(out=rl[:RPC, :nw], in0=s[:RPC, :nw],
                                    in1=rl[:RPC, :nw], op=ALU.mult)
            nc.vector.tensor_tensor(out=ho_b[b][:RPC, n0:n0 + nw], in0=rl[:RPC, :nw],
                                    in1=pT[:RPC, n0:n0 + nw], op=ALU.add)
    nc.tensor.matmul(out=lps[:RPC, :NCLS], lhsT=onesb_sb[:1, :RPC],
                     rhs=linb_sb[:1, :], start=True, stop=False)
    keep_warm(0, 14)
    for nh, (n0, nw) in enumerate(NHW):
        for g in range(4 * nh, 4 * nh + 4):
            g0, mg = GCH[g]
            for b, (c0_, c1_) in ((0, (0, 8)), (1, (8, RPC))):
                tp2 = ps_cv.tile([128, 512], F32, tag="cv", name=f"tpho{g}_{b}")
                nc.tensor.matmul(out=tp2[:mg, :RPC], lhsT=ho_b[b][:, g0:g0 + mg],
                                 rhs=id_b[:RPC, :RPC], start=True, stop=True)
                nc.scalar.activation(out=hoT[g][:mg, c0_:c1_], in_=tp2[:mg, c0_:c1_],
                                     func=AFT.Copy)
            nc.tensor.matmul(out=lps[:RPC, :NCLS], lhsT=hoT[g][:mg, :RPC],
                             rhs=lint_sb[:mg, g, :], start=False, stop=(g == 7))
    outn = small.tile([RPC, NCLS], F32, tag="outn")
    nc.vector.tensor_copy(out=outn[:], in_=lps[:RPC, :NCLS])
    nc.sync.dma_start(out=d_res[:], in_=outn[:])
    ctx.close()


def _pack_inputs(inputs):
    """Host-side packing: per-core index slices + shared packed weight arrays."""
    f32 = np.float32
    bf16 = ml_dtypes.bfloat16
    x = np.asarray(inputs["x"]).astype(np.int32)                  # [64, 128]
    wfull = {"p": np.zeros((KMAX, E, F), f32), "r": np.zeros((KMAX, E, F), f32)}
    offs = [0, 300, 600]
    for ui, (f, n) in enumerate(zip(FS, NF)):
        o = offs[ui]
        cw = np.asarray(inputs[f"conv_w{f}"], f32)                # [f, E, n]
        cwr = np.asarray(inputs[f"convr_w{f}"], f32)
        for k in range(f):
            wfull["p"][k, :, o:o + n] = cw[k]
            wfull["r"][k, :, o:o + n] = cwr[k]
    fp8 = ml_dtypes.float8_e4m3
    shared = {}
    for br in "pr":
        w = wfull[br]
        w2 = np.zeros((128, KMAX, 2, PF), f32)
        for k in range(KMAX):
            w2[:, k, 0, :F] = w[k, 0:128, :]
            w2[:, k, 1, :F] = w[k, 128:256, :]
        shared[f"w2_{br}"] = w2.astype(fp8)
        rem2 = np.zeros((108, 2, PF), f32)
        for t_ in range(2):
            for half in range(2):
                sh = 2 * t_ + half
                rem2[64 * half:64 * half + 44, t_, :F] = w[sh, 256:300, :]
        shared[f"rem2_{br}"] = rem2.astype(fp8)
        rc = np.zeros((44, PF), f32)
        rc[:, :F] = w[4, 256:300, :]
        shared[f"rc_{br}"] = rc.astype(fp8)
    cbf = np.stack([
        np.concatenate([np.asarray(inputs[f"conv_b{f}"], f32) for f in FS]),
        np.concatenate([np.asarray(inputs[f"convr_b{f}"], f32) for f in FS]),
    ], axis=1)                                                    # [1000, 2]
    cb = np.zeros((128, 8, 2), f32)
    lintf = np.asarray(inputs["lin_w"], f32).T                    # [1000, 2]
    lint = np.zeros((128, 8, NCLS), f32)
    for g in range(8):
        mg = min(128, F - g * 128)
        cb[:mg, g, :] = cbf[g * 128:g * 128 + mg, :]
        lint[:mg, g, :] = lintf[g * 128:g * 128 + mg, :]
    hwtT = np.stack([np.asarray(inputs["hw_w"], f32).T,
                     np.asarray(inputs["hwr_w"], f32).T])          # [2, F, F]
    hwt = np.zeros((8, 128, 2 * F), f32)
    for g in range(8):
        mg = min(128, F - g * 128)
        hwt[g, :mg, :F] = hwtT[0, g * 128:g * 128 + mg, :]
        hwt[g, :mg, F:] = hwtT[1, g * 128:g * 128 + mg, :]
    hwt[7, 104, :F] = np.asarray(inputs["hw_b"], f32)
    hwt[7, 104, F:] = np.asarray(inputs["hwr_b"], f32)
    hwt = hwt.astype(bf16)
    hwb = np.stack([np.asarray(inputs["hw_b"], f32),
                    np.asarray(inputs["hwr_b"], f32)])            # [2, 1000]
    shared.update(
        emb=np.ascontiguousarray(np.asarray(inputs["emb_w"], f32)).astype(bf16),
        hwt=hwt, cb=cb,
        lint=lint.astype(bf16),
        linb=np.asarray(inputs["lin_b"], f32).reshape(1, NCLS).astype(bf16),
        identb=np.eye(128, dtype=f32).astype(bf16),
        onesb=np.ones((1, RPC), bf16),
        hwbb=hwb.reshape(1, 2 * F).astype(bf16))
    in_maps = []
    for c in range(N_CORES):
        rows = list(range(8 * c, 8 * c + 8)) + [2 * c, 2 * c + 1]
        idx = np.ascontiguousarray(x[rows].T)                     # [128, 10]
        in_maps.append(dict(idx=idx, **shared))
    return in_maps


def run_cores(inputs, trace=False, **kw):
    """Compile (cached) and run on 8 cores; returns (per-core results, BassKernelResults)."""
    if "nc" not in _CACHE:
        _CACHE["nc"] = _build_program()
    nc = _CACHE["nc"]
    in_maps = _pack_inputs(inputs)
    res = run_bass_kernel_spmd(nc, in_maps, list(range(N_CORES)), trace=trace, **kw)
    return res.results, res


def combine(results) -> np.ndarray:
    """Host epilogue: per-row log-softmax + L2-normalize, then the cosine sum."""
    logits = np.concatenate([results[c]["res"] for c in range(N_CORES)]).astype(np.float64)
    m = logits.max(axis=1, keepdims=True)
    ls = m + np.log(np.exp(logits - m).sum(axis=1, keepdims=True))
    pred = logits - ls                                            # [80, 2]
    n = np.maximum(np.linalg.norm(pred, axis=1, keepdims=True), 1e-8)
    pn = pred / n
    is_pred = np.tile([True] * 8 + [False] * 2, N_CORES)
    p, r = pn[is_pred], pn[~is_pred]
    values = p @ r.sum(axis=0)
    return np.log(values / values.sum()).astype(np.float32)


def kernel(**inputs) -> np.ndarray:
    results, _ = run_cores(inputs)
    return combine(results)
